# revision 1
# baseline (speedup 1.0000x reference)
import numpy as np
import jax
import jax.numpy as jnp

# AFSIFormer forward, data-parallel over batch across 8 NeuronCores.
# Shapes (hardcoded): x (8, 256, 128, 128) f32; DIM=256, C2=128, HEADS=8.

HEADS = 8


def _conv2d(x, w, b, stride, pad, groups=1):
    y = jax.lax.conv_general_dilated(
        x, w, (stride, stride), [(pad, pad), (pad, pad)],
        dimension_numbers=('NCHW', 'OIHW', 'NCHW'),
        feature_group_count=groups)
    return y + b[None, :, None, None]


def _dct_matrix(M):
    n = np.arange(M)
    D = np.cos(np.pi * (2 * n[None, :] + 1) * n[:, None] / (2 * M)) * np.sqrt(2.0 / M)
    D[0] *= np.sqrt(0.5)
    return jnp.asarray(D, jnp.float32)


def _rel_position_index():
    c = np.stack(np.meshgrid(np.arange(4), np.arange(4), indexing='ij')).reshape(2, -1)
    rc = (c[:, :, None] - c[:, None, :]).transpose(1, 2, 0)
    rc[:, :, 0] += 3
    rc[:, :, 1] += 3
    rc[:, :, 0] *= 7
    return rc.sum(-1)


_REL_IDX = _rel_position_index()


def _afsi_attention(x, p, use_rpb=False):
    B, N, C = x.shape
    hd = C // HEADS
    qkv = (x @ p['qkv_w'].T + p['qkv_b']).reshape(B, N, 3, HEADS, hd).transpose(2, 0, 3, 1, 4)
    q, k, v = qkv[0] * (hd ** -0.5), qkv[1], qkv[2]
    attn = jnp.einsum('bhnd,bhmd->bhnm', q, k)
    if use_rpb:
        bias = p['rpb'][_REL_IDX.reshape(-1)].reshape(16, 16, HEADS).transpose(2, 0, 1)
        attn = attn + bias[None]
    x_mean = x.mean(-1)
    M = int(np.ceil(np.sqrt(N)))
    D = _dct_matrix(M)
    padded = jnp.zeros((B, M * M), x.dtype).at[:, :N].set(x_mean).reshape(B, M, M)
    dct = jnp.einsum('ij,bjk,lk->bil', D, padded, D).reshape(B, M * M)[:, :N]
    dct = jnp.clip(dct, -10.0, 10.0)
    dct = dct / (jnp.linalg.norm(dct, axis=1, keepdims=True) + 1e-5)
    hw = jax.nn.relu(x.mean(1) @ p['h1_w'].T + p['h1_b']) @ p['h2_w'].T + p['h2_b']
    wdct = dct[:, None, :] * hw[:, :, None]
    freq = jnp.einsum('bhn,bhm->bnm', wdct, wdct)
    freq = jnp.clip(freq / jnp.maximum(freq.sum(-1, keepdims=True), 1e-5), 0.0, 1.0)
    a = jax.nn.sigmoid(p['freq_weight'])
    attn = jax.nn.softmax((1 - a) * attn + a * freq[:, None], -1)
    out = jnp.einsum('bhnm,bhmd->bhnd', attn, v).transpose(0, 2, 1, 3).reshape(B, N, C)
    return out @ p['proj_w'].T + p['proj_b']


def _layer_norm(x, p):
    mu = x.mean(-1, keepdims=True)
    var = ((x - mu) ** 2).mean(-1, keepdims=True)
    return (x - mu) / jnp.sqrt(var + 1e-5) * p['g'] + p['b']


def _mlp(xl, p, H, W):
    B, L, C = xl.shape
    xc = xl.reshape(B, H, W, C).transpose(0, 3, 1, 2)
    xcat = jnp.concatenate([_conv2d(xc, p['w1'], p['b1'], 1, 0),
                            _conv2d(xc, p['w2'], p['b2'], 1, 1),
                            _conv2d(xc, p['w3'], p['b3'], 1, 3)], 1)
    h = jax.nn.gelu(_conv2d(xcat, p['f1'], p['f1b'], 1, 0), approximate=False)
    h = jax.nn.gelu(_conv2d(h, p['f2'], p['f2b'], 1, 0), approximate=False)
    return h.transpose(0, 2, 3, 1).reshape(B, L, C)


def _forward(x, down_p, hconv_p, wconv_p, badam_p, lwam_p, norm_p, mlp_p, out_p):
    xd = _conv2d(x, down_p['w'], down_p['b'], 2, 0, groups=32)
    xd = xd * (down_p['gamma'] / jnp.sqrt(1.0 + 1e-5))[None, :, None, None] \
        + down_p['beta'][None, :, None, None]
    B, C2, H, W = xd.shape
    xb = xd.transpose(0, 2, 3, 1)
    win = xb.reshape(B, H // 4, 4, W // 4, 4, C2).transpose(0, 1, 3, 2, 4, 5).reshape(-1, 16, C2)
    local = _afsi_attention(win, lwam_p, use_rpb=True)
    local = local.reshape(B, H // 4, W // 4, 4, 4, C2).transpose(0, 1, 3, 2, 4, 5).reshape(B, H, W, C2)
    xg = _conv2d(_conv2d(xd, hconv_p['w'], hconv_p['b'], 2, 0), wconv_p['w'], wconv_p['b'], 2, 0)
    g = _afsi_attention(xg.transpose(0, 2, 3, 1).reshape(B, (H // 4) * (W // 4), C2), badam_p)
    g = jnp.repeat(jnp.repeat(g.reshape(B, H // 4, W // 4, C2), 4, axis=1), 4, axis=2)
    yl = (local + g).reshape(B, H * W, C2)
    yl = yl + _mlp(_layer_norm(yl, norm_p), mlp_p, H, W)
    yb = yl.reshape(B, H, W, C2).transpose(0, 3, 1, 2)
    out = _conv2d(yb, out_p['w'], out_p['b'], 1, 1, groups=32)
    out = jnp.repeat(jnp.repeat(out, 2, axis=2), 2, axis=3)
    return x + out


def _to_jnp_tree(t):
    if isinstance(t, dict):
        return {k: _to_jnp_tree(v) for k, v in t.items()}
    return jnp.asarray(np.asarray(t), jnp.float32)


_COMPILED = {}


def kernel(x, down_p, hconv_p, wconv_p, badam_p, lwam_p, norm_p, mlp_p, out_p):
    params = _to_jnp_tree(dict(down_p=down_p, hconv_p=hconv_p, wconv_p=wconv_p,
                               badam_p=badam_p, lwam_p=lwam_p, norm_p=norm_p,
                               mlp_p=mlp_p, out_p=out_p))
    x = np.asarray(x, np.float32)
    B = x.shape[0]

    devs = jax.devices()
    n = min(len(devs), B)
    if B % n != 0:
        n = 1

    if 'fn' not in _COMPILED:
        def _per_device(xs, params):
            return _forward(xs, params['down_p'], params['hconv_p'], params['wconv_p'],
                            params['badam_p'], params['lwam_p'], params['norm_p'],
                            params['mlp_p'], params['out_p'])
        try:
            fn = jax.pmap(_per_device, axis_name='b', devices=devs[:n],
                          in_axes=(0, None), out_axes=0)
            xs = x.reshape(n, B // n, *x.shape[1:])
            out = np.asarray(fn(xs, params))
            out = out.reshape(B, *out.shape[2:])
            _COMPILED['fn'] = ('pmap', fn, n)
            return out.astype(np.float32)
        except Exception:
            _COMPILED['fn'] = ('cpu', None, 1)

    kind, fn, n = _COMPILED['fn']
    if kind == 'pmap':
        xs = x.reshape(n, B // n, *x.shape[1:])
        out = np.asarray(fn(xs, params))
        return out.reshape(B, *out.shape[2:]).astype(np.float32)

    with jax.default_device(jax.devices('cpu')[0]):
        out = _forward(jnp.asarray(x), params['down_p'], params['hconv_p'],
                       params['wconv_p'], params['badam_p'], params['lwam_p'],
                       params['norm_p'], params['mlp_p'], params['out_p'])
        return np.asarray(out, np.float32)


if __name__ == '__main__':
    pass


# revision 4
# speedup vs baseline: 48.7948x; 48.7948x over previous
import numpy as np
import jax
import jax.numpy as jnp

# AFSIFormer forward, data-parallel over batch across 8 NeuronCores.
# Shapes (hardcoded): x (8, 256, 128, 128) f32; DIM=256, C2=128, HEADS=8.

HEADS = 8


def _conv2d(x, w, b, stride, pad, groups=1, fast=True):
    # Heavy convs run in bf16 on the PE array (4x faster than fp32); bias add
    # and everything downstream stays f32. Accumulation is f32 in hardware.
    if fast:
        x = x.astype(jnp.bfloat16)
        w = w.astype(jnp.bfloat16)
    y = jax.lax.conv_general_dilated(
        x, w, (stride, stride), [(pad, pad), (pad, pad)],
        dimension_numbers=('NCHW', 'OIHW', 'NCHW'),
        feature_group_count=groups,
        preferred_element_type=jnp.float32)
    return y.astype(jnp.float32) + b[None, :, None, None]


def _bmm16(a, b_):
    # bf16 matmul with f32 accumulate/output
    return jnp.matmul(a.astype(jnp.bfloat16), b_.astype(jnp.bfloat16),
                      preferred_element_type=jnp.float32).astype(jnp.float32)


def _dct_matrix(M):
    n = np.arange(M)
    D = np.cos(np.pi * (2 * n[None, :] + 1) * n[:, None] / (2 * M)) * np.sqrt(2.0 / M)
    D[0] *= np.sqrt(0.5)
    return jnp.asarray(D, jnp.float32)


def _rel_position_index():
    c = np.stack(np.meshgrid(np.arange(4), np.arange(4), indexing='ij')).reshape(2, -1)
    rc = (c[:, :, None] - c[:, None, :]).transpose(1, 2, 0)
    rc[:, :, 0] += 3
    rc[:, :, 1] += 3
    rc[:, :, 0] *= 7
    return rc.sum(-1)


_REL_IDX = _rel_position_index()


def _afsi_attention(x, p, use_rpb=False):
    B, N, C = x.shape
    hd = C // HEADS
    qkv = (_bmm16(x, p['qkv_w'].T) + p['qkv_b']).reshape(B, N, 3, HEADS, hd).transpose(2, 0, 3, 1, 4)
    q, k, v = qkv[0] * (hd ** -0.5), qkv[1], qkv[2]
    attn = jnp.einsum('bhnd,bhmd->bhnm', q.astype(jnp.bfloat16), k.astype(jnp.bfloat16),
                      preferred_element_type=jnp.float32).astype(jnp.float32)
    if use_rpb:
        bias = p['rpb'][_REL_IDX.reshape(-1)].reshape(16, 16, HEADS).transpose(2, 0, 1)
        attn = attn + bias[None]
    x_mean = x.mean(-1)
    M = int(np.ceil(np.sqrt(N)))
    D = _dct_matrix(M)
    padded = jnp.zeros((B, M * M), x.dtype).at[:, :N].set(x_mean).reshape(B, M, M)
    dct = jnp.einsum('ij,bjk,lk->bil', D, padded, D).reshape(B, M * M)[:, :N]
    dct = jnp.clip(dct, -10.0, 10.0)
    dct = dct / (jnp.linalg.norm(dct, axis=1, keepdims=True) + 1e-5)
    hw = jax.nn.relu(x.mean(1) @ p['h1_w'].T + p['h1_b']) @ p['h2_w'].T + p['h2_b']
    wdct = dct[:, None, :] * hw[:, :, None]
    freq = jnp.einsum('bhn,bhm->bnm', wdct, wdct)
    freq = jnp.clip(freq / jnp.maximum(freq.sum(-1, keepdims=True), 1e-5), 0.0, 1.0)
    a = jax.nn.sigmoid(p['freq_weight'])
    attn = jax.nn.softmax((1 - a) * attn + a * freq[:, None], -1)
    out = jnp.einsum('bhnm,bhmd->bhnd', attn.astype(jnp.bfloat16), v.astype(jnp.bfloat16),
                     preferred_element_type=jnp.float32).astype(jnp.float32)
    out = out.transpose(0, 2, 1, 3).reshape(B, N, C)
    return _bmm16(out, p['proj_w'].T) + p['proj_b']


def _layer_norm(x, p):
    mu = x.mean(-1, keepdims=True)
    var = ((x - mu) ** 2).mean(-1, keepdims=True)
    return (x - mu) / jnp.sqrt(var + 1e-5) * p['g'] + p['b']


def _mlp(xl, p, H, W):
    B, L, C = xl.shape
    xc = xl.reshape(B, H, W, C).transpose(0, 3, 1, 2)
    xcat = jnp.concatenate([_conv2d(xc, p['w1'], p['b1'], 1, 0),
                            _conv2d(xc, p['w2'], p['b2'], 1, 1),
                            _conv2d(xc, p['w3'], p['b3'], 1, 3)], 1)
    h = jax.nn.gelu(_conv2d(xcat, p['f1'], p['f1b'], 1, 0), approximate=False)
    h = jax.nn.gelu(_conv2d(h, p['f2'], p['f2b'], 1, 0), approximate=False)
    return h.transpose(0, 2, 3, 1).reshape(B, L, C)


def _forward(x, down_p, hconv_p, wconv_p, badam_p, lwam_p, norm_p, mlp_p, out_p):
    xd = _conv2d(x, down_p['w'], down_p['b'], 2, 0, groups=32)
    xd = xd * (down_p['gamma'] / jnp.sqrt(1.0 + 1e-5))[None, :, None, None] \
        + down_p['beta'][None, :, None, None]
    B, C2, H, W = xd.shape
    xb = xd.transpose(0, 2, 3, 1)
    win = xb.reshape(B, H // 4, 4, W // 4, 4, C2).transpose(0, 1, 3, 2, 4, 5).reshape(-1, 16, C2)
    local = _afsi_attention(win, lwam_p, use_rpb=True)
    local = local.reshape(B, H // 4, W // 4, 4, 4, C2).transpose(0, 1, 3, 2, 4, 5).reshape(B, H, W, C2)
    xg = _conv2d(_conv2d(xd, hconv_p['w'], hconv_p['b'], 2, 0), wconv_p['w'], wconv_p['b'], 2, 0)
    g = _afsi_attention(xg.transpose(0, 2, 3, 1).reshape(B, (H // 4) * (W // 4), C2), badam_p)
    g = jnp.repeat(jnp.repeat(g.reshape(B, H // 4, W // 4, C2), 4, axis=1), 4, axis=2)
    yl = (local + g).reshape(B, H * W, C2)
    yl = yl + _mlp(_layer_norm(yl, norm_p), mlp_p, H, W)
    yb = yl.reshape(B, H, W, C2).transpose(0, 3, 1, 2)
    out = _conv2d(yb, out_p['w'], out_p['b'], 1, 1, groups=32)
    out = jnp.repeat(jnp.repeat(out, 2, axis=2), 2, axis=3)
    return x + out


def _to_jnp_tree(t):
    if isinstance(t, dict):
        return {k: _to_jnp_tree(v) for k, v in t.items()}
    return jnp.asarray(np.asarray(t), jnp.float32)


_COMPILED = {}


def kernel(x, down_p, hconv_p, wconv_p, badam_p, lwam_p, norm_p, mlp_p, out_p):
    params = _to_jnp_tree(dict(down_p=down_p, hconv_p=hconv_p, wconv_p=wconv_p,
                               badam_p=badam_p, lwam_p=lwam_p, norm_p=norm_p,
                               mlp_p=mlp_p, out_p=out_p))
    x = np.asarray(x, np.float32)
    B = x.shape[0]

    devs = jax.devices()
    n = min(len(devs), B)
    if B % n != 0:
        n = 1

    if 'fn' not in _COMPILED:
        def _per_device(xs, params):
            return _forward(xs, params['down_p'], params['hconv_p'], params['wconv_p'],
                            params['badam_p'], params['lwam_p'], params['norm_p'],
                            params['mlp_p'], params['out_p'])
        try:
            fn = jax.pmap(_per_device, axis_name='b', devices=devs[:n],
                          in_axes=(0, None), out_axes=0)
            xs = x.reshape(n, B // n, *x.shape[1:])
            out = np.asarray(fn(xs, params))
            out = out.reshape(B, *out.shape[2:])
            _COMPILED['fn'] = ('pmap', fn, n)
            return out.astype(np.float32)
        except Exception:
            _COMPILED['fn'] = ('cpu', None, 1)

    kind, fn, n = _COMPILED['fn']
    if kind == 'pmap':
        xs = x.reshape(n, B // n, *x.shape[1:])
        out = np.asarray(fn(xs, params))
        return out.reshape(B, *out.shape[2:]).astype(np.float32)

    with jax.default_device(jax.devices('cpu')[0]):
        out = _forward(jnp.asarray(x), params['down_p'], params['hconv_p'],
                       params['wconv_p'], params['badam_p'], params['lwam_p'],
                       params['norm_p'], params['mlp_p'], params['out_p'])
        return np.asarray(out, np.float32)


if __name__ == '__main__':
    pass
